# revision 3
# baseline (speedup 1.0000x reference)
"""Trainium2 Bass kernel for the BiDAF-style attention layer.

Math (per batch b, sentence s):
  logits[p,q] = h.w_h (hs) + u.w_u (us) + (h*w_hu).u + b  (+ mask NEG terms)
  c2q  = softmax_q(logits);      u_a = c2q @ u
  q2c  = softmax_p(max_q logits); h_a = q2c @ h
  g    = concat([h, u_a, h*u_a, h*h_a], -1)

Strategy: data-parallel over B across 8 cores (no collectives). Everything
on-device lives in a d-on-partitions ("transposed") layout so the logits
matmul needs no on-chip transposes of h:
  - host feeds hT = h[b]^T as [S, D, P] (d contiguous -> partition dim)
  - logits computed as MT[q,p] (q on partitions, p on free dim)
  - output written transposed as [S, 4D, P]; host transposes back.
b is dropped entirely (softmax shift invariance); us/u_mask are folded into
the logits matmul as a K=1 accumulation row; w_h is folded as an extra
output row of the same matmul (giving hs for free).
"""

import os
import sys

import numpy as np

for _p in ("/opt/trn_rl_repo",):
    if _p not in sys.path and os.path.isdir(_p):
        sys.path.append(_p)

B, S, P, Q, D = 8, 16, 256, 96, 768
NCORES = 8
C = D // 128  # 6 d-chunks
NEG = 1e30

_NC = None
_TRACE = False
LAST_EXEC_NS = None


def _build_nc():
    import concourse.bacc as bacc
    import concourse.tile as tile
    from concourse import mybir

    f32 = mybir.dt.float32
    bf16 = mybir.dt.bfloat16
    AF = mybir.ActivationFunctionType
    ALU = mybir.AluOpType
    AX = mybir.AxisListType

    nc = bacc.Bacc(None, target_bir_lowering=False)

    ht = nc.declare_dram_parameter("ht", [S, D, P], f32, isOutput=False)
    uwt = nc.declare_dram_parameter("uwt", [D, Q + 1], f32, isOutput=False)
    usm = nc.declare_dram_parameter("usm", [1, Q + 1], f32, isOutput=False)
    uu = nc.declare_dram_parameter("u", [Q, D], f32, isOutput=False)
    hmf = nc.declare_dram_parameter("hmneg", [1, S * P], f32, isOutput=False)
    idn = nc.declare_dram_parameter("ident", [128, 128], f32, isOutput=False)
    out = nc.declare_dram_parameter("out", [S, 4 * D, P], f32, isOutput=True)

    with tile.TileContext(nc) as tc:
        with (
            tc.tile_pool(name="singles", bufs=1) as singles,
            tc.tile_pool(name="ht_pool", bufs=3) as ht_pool,
            tc.tile_pool(name="e_pool", bufs=3) as e_pool,
            tc.tile_pool(name="c2q_pool", bufs=3) as c2q_pool,
            tc.tile_pool(name="g2_pool", bufs=3) as g2_pool,
            tc.tile_pool(name="g3_pool", bufs=3) as g3_pool,
            tc.tile_pool(name="g4_pool", bufs=3) as g4_pool,
            tc.tile_pool(name="tmp_pool", bufs=2) as tmp_pool,
            tc.tile_pool(name="sm_pool", bufs=4) as sm,
            tc.tile_pool(name="ps_mt", bufs=2, space="PSUM") as ps_mt,
            tc.tile_pool(name="ps_sm", bufs=3, space="PSUM") as ps_sm,
            tc.tile_pool(name="ps_ua", bufs=1, space="PSUM") as ps_ua,
        ):
            # ---- per-core statics ----
            ones_f = singles.tile([1, 256], f32)
            nc.vector.memset(ones_f, 1.0)
            ones_bf = singles.tile([128, 1], bf16)
            nc.vector.memset(ones_bf, 1.0)
            ident_f = singles.tile([128, 128], f32)
            nc.sync.dma_start(out=ident_f, in_=idn[:, :])
            ident_bf = singles.tile([128, 128], bf16)
            nc.vector.tensor_copy(ident_bf, ident_f)
            uwt_sb = singles.tile([128, C, Q + 1], f32)
            nc.sync.dma_start(
                out=uwt_sb, in_=uwt.rearrange("(c p) q -> p c q", p=128)
            )
            usm_sb = singles.tile([1, Q + 1], f32)
            nc.sync.dma_start(out=usm_sb, in_=usm[:, :])
            u_f = singles.tile([Q, D], f32)
            nc.sync.dma_start(out=u_f, in_=uu[:, :])
            u_bf = singles.tile([Q, D], bf16)
            nc.vector.tensor_copy(u_bf, u_f)
            hm_sb = singles.tile([1, S * P], f32)
            nc.sync.dma_start(out=hm_sb, in_=hmf[:, :])

            for s in range(S):
                # ---- load hT[s]: [768,256] -> [128, 6, 256] (d on partitions)
                ht_sb = ht_pool.tile([128, C, 256], f32)
                nc.sync.dma_start(
                    out=ht_sb, in_=ht[s].rearrange("(c p) q -> p c q", p=128)
                )

                # ---- logits MT_ext [97, 256]: rows 0:96 = logits+usm, row 96 = hs
                mt = ps_mt.tile([Q + 1, 256], f32, tag="psmt")
                for c in range(C):
                    nc.tensor.matmul(
                        mt,
                        lhsT=uwt_sb[:, c, :],
                        rhs=ht_sb[:, c, :],
                        start=(c == 0),
                        stop=False,
                    )
                nc.tensor.matmul(
                    mt, lhsT=usm_sb, rhs=ones_f[:, 0:256], start=False, stop=True
                )

                # ---- E = exp(logits) [96,256] bf16; hs row -> sbuf
                e_sb = e_pool.tile([Q, 256], bf16)
                nc.scalar.activation(e_sb, mt[0:Q, :], AF.Exp)
                hs_row = sm.tile([1, 256], f32)
                nc.scalar.copy(hs_row, mt[Q : Q + 1, :])

                # ---- Zq[p] = sum_q E   (ones matmul), then 1/Zq via exp(-ln)
                zq = ps_sm.tile([1, 256], f32, tag="pssm")
                nc.tensor.matmul(zq, lhsT=ones_bf[0:Q, :], rhs=e_sb)
                lnzq = sm.tile([1, 256], f32)
                nc.scalar.activation(lnzq, zq, AF.Ln)
                zqr = sm.tile([1, 256], f32)
                nc.scalar.activation(zqr, lnzq, AF.Exp, scale=-1.0)

                # ---- broadcast 1/Zq over q partitions; c2q = E * (1/Zq)
                zb = ps_sm.tile([Q, 256], f32, tag="pssm")
                nc.tensor.matmul(zb, lhsT=ones_f[0:1, 0:Q], rhs=zqr)
                c2q = c2q_pool.tile([Q, 256], bf16)
                nc.vector.tensor_mul(c2q, e_sb, zb)

                # ---- u_aT[d,p] = sum_q u[q,d] c2q[q,p]  -> psum [128, 6, 256]
                ua = ps_ua.tile([128, C, 256], f32)
                for c in range(C):
                    nc.tensor.matmul(
                        ua[:, c, :],
                        lhsT=u_bf[:, c * 128 : (c + 1) * 128],
                        rhs=c2q,
                    )
                g2 = g2_pool.tile([128, C, 256], f32)
                nc.scalar.copy(g2, ua)
                g3 = g3_pool.tile([128, C, 256], f32)
                nc.vector.tensor_mul(g3, ht_sb, g2)

                # ---- rmax path: transpose E halves -> [128, 2, 96], max over q
                te = ps_mt.tile([128, 2, Q], bf16, tag="psmt")
                nc.tensor.transpose(
                    te[:, 0, :], e_sb[:, 0:128], ident_bf[0:Q, 0:Q]
                )
                nc.tensor.transpose(
                    te[:, 1, :], e_sb[:, 128:256], ident_bf[0:Q, 0:Q]
                )
                m_col2 = sm.tile([128, 2], f32)
                nc.vector.tensor_reduce(m_col2, te, axis=AX.X, op=ALU.max)
                mrow = ps_sm.tile([1, 256], f32, tag="pssm")
                nc.tensor.transpose(
                    mrow[0:1, 0:128], m_col2[:, 0:1], ident_f
                )
                nc.tensor.transpose(
                    mrow[0:1, 128:256], m_col2[:, 1:2], ident_f
                )

                # ---- q2c over p (single-partition row ops)
                t0 = sm.tile([1, 256], f32)
                nc.vector.tensor_add(
                    t0, hs_row, hm_sb[:, s * 256 : (s + 1) * 256]
                )
                xrow = sm.tile([1, 256], f32)
                nc.scalar.activation(xrow, t0, AF.Exp)
                erow = sm.tile([1, 256], f32)
                nc.vector.tensor_mul(erow, xrow, mrow)
                zp = sm.tile([1, 1], f32)
                nc.vector.tensor_reduce(zp, erow, axis=AX.X, op=ALU.add)
                rzp = sm.tile([1, 1], f32)
                nc.vector.reciprocal(rzp, zp)
                q2c = sm.tile([1, 256], f32)
                nc.vector.tensor_scalar_mul(q2c, in0=erow, scalar1=rzp)

                # ---- h_aT[d] = sum_p hT[d,p] q2c[p]
                qb = ps_sm.tile([128, 256], f32, tag="pssm")
                nc.tensor.matmul(qb, lhsT=ones_f[0:1, 0:128], rhs=q2c)
                qb_sb = sm.tile([128, 256], f32)
                nc.scalar.copy(qb_sb, qb)
                tmp = tmp_pool.tile([128, C, 256], f32)
                for c in range(C):
                    nc.gpsimd.tensor_mul(tmp[:, c, :], ht_sb[:, c, :], qb_sb)
                ha_col = sm.tile([128, C], f32)
                nc.vector.tensor_reduce(ha_col, tmp, axis=AX.X, op=ALU.add)

                # ---- g4 = hT * h_a (per-partition scalar per chunk)
                g4 = g4_pool.tile([128, C, 256], f32)
                for c in range(C):
                    nc.vector.tensor_scalar_mul(
                        g4[:, c, :],
                        in0=ht_sb[:, c, :],
                        scalar1=ha_col[:, c : c + 1],
                    )

                # ---- outputs
                def oview(k):
                    return out[s, k * D : (k + 1) * D, :].rearrange(
                        "(c p) q -> p c q", p=128
                    )

                nc.sync.dma_start(out=oview(0), in_=ht_sb)
                nc.sync.dma_start(out=oview(1), in_=g2)
                nc.sync.dma_start(out=oview(2), in_=g3)
                nc.sync.dma_start(out=oview(3), in_=g4)

    nc.compile()
    return nc


def _get_nc():
    global _NC
    if _NC is None:
        _NC = _build_nc()
    return _NC


def kernel(h, u, h_mask, u_mask, is_train=0, w=None, b=None):
    global LAST_EXEC_NS
    h = np.asarray(h, dtype=np.float32)
    u = np.asarray(u, dtype=np.float32)
    h_mask = np.asarray(h_mask, dtype=np.float32)
    u_mask = np.asarray(u_mask, dtype=np.float32)
    w = np.asarray(w, dtype=np.float32)

    w_h, w_u, w_hu = w[:D], w[D : 2 * D], w[2 * D :]

    # host-side prep (tiny)
    hT = np.ascontiguousarray(h.transpose(0, 1, 3, 2))  # [B,S,D,P]
    uw = u * w_hu[None, None, :]  # [B,Q,D]
    uwt = np.empty((B, D, Q + 1), dtype=np.float32)
    uwt[:, :, :Q] = uw.transpose(0, 2, 1)
    uwt[:, :, Q] = w_h[None, :]
    usm = np.zeros((B, 1, Q + 1), dtype=np.float32)
    usm[:, 0, :Q] = u @ w_u + (u_mask - 1.0) * NEG
    hmneg = ((h_mask - 1.0) * NEG).reshape(B, 1, S * P).astype(np.float32)
    ident = np.eye(128, dtype=np.float32)

    in_maps = [
        {
            "ht": hT[i],
            "uwt": uwt[i],
            "usm": usm[i],
            "u": u[i],
            "hmneg": hmneg[i],
            "ident": ident,
        }
        for i in range(NCORES)
    ]

    from concourse.bass_utils import run_bass_kernel_spmd

    nc = _get_nc()
    res = run_bass_kernel_spmd(
        nc, in_maps, core_ids=list(range(NCORES)), trace=_TRACE
    )
    LAST_EXEC_NS = res.exec_time_ns

    gT = np.stack([res.results[i]["out"] for i in range(NCORES)])  # [B,S,4D,P]
    g = np.ascontiguousarray(gT.transpose(0, 1, 3, 2))  # [B,S,P,4D]
    return g


# revision 4
# speedup vs baseline: 1.0044x; 1.0044x over previous
"""Trainium2 Bass kernel for the BiDAF-style attention layer.

Math (per batch b, sentence s):
  logits[p,q] = h.w_h (hs) + u.w_u (us) + (h*w_hu).u + b  (+ mask NEG terms)
  c2q  = softmax_q(logits);      u_a = c2q @ u
  q2c  = softmax_p(max_q logits); h_a = q2c @ h
  g    = concat([h, u_a, h*u_a, h*h_a], -1)

Strategy: data-parallel over B across 8 cores (no collectives). Everything
on-device lives in a d-on-partitions ("transposed") layout so the logits
matmul needs no on-chip transposes of h:
  - host feeds hT = h[b]^T as [S, D, P] (d contiguous -> partition dim)
  - logits computed as MT[q,p] (q on partitions, p on free dim)
  - output written transposed as [S, 4D, P]; host transposes back.
b is dropped entirely (softmax shift invariance); us/u_mask are folded into
the logits matmul as a K=1 accumulation row; w_h is folded as an extra
output row of the same matmul (giving hs for free).
"""

import os
import sys

import numpy as np

for _p in ("/opt/trn_rl_repo",):
    if _p not in sys.path and os.path.isdir(_p):
        sys.path.append(_p)

B, S, P, Q, D = 8, 16, 256, 96, 768
NCORES = 8
C = D // 128  # 6 d-chunks
NEG = 1e30

_NC = None
_TRACE = False
LAST_EXEC_NS = None


def _build_nc():
    import concourse.bacc as bacc
    import concourse.tile as tile
    from concourse import mybir

    f32 = mybir.dt.float32
    bf16 = mybir.dt.bfloat16
    AF = mybir.ActivationFunctionType
    ALU = mybir.AluOpType
    AX = mybir.AxisListType

    nc = bacc.Bacc(None, target_bir_lowering=False)

    ht = nc.declare_dram_parameter("ht", [S, D, P], f32, isOutput=False)
    uwt = nc.declare_dram_parameter("uwt", [D, Q + 1], f32, isOutput=False)
    usm = nc.declare_dram_parameter("usm", [1, Q + 1], f32, isOutput=False)
    uu = nc.declare_dram_parameter("u", [Q, D], f32, isOutput=False)
    hmf = nc.declare_dram_parameter("hmneg", [1, S * P], f32, isOutput=False)
    idn = nc.declare_dram_parameter("ident", [128, 128], f32, isOutput=False)
    out = nc.declare_dram_parameter("out", [S, 4 * D, P], f32, isOutput=True)

    with tile.TileContext(nc) as tc:
        with (
            tc.tile_pool(name="singles", bufs=1) as singles,
            tc.tile_pool(name="ht_pool", bufs=3) as ht_pool,
            tc.tile_pool(name="e_pool", bufs=3) as e_pool,
            tc.tile_pool(name="c2q_pool", bufs=3) as c2q_pool,
            tc.tile_pool(name="g2_pool", bufs=3) as g2_pool,
            tc.tile_pool(name="g3_pool", bufs=3) as g3_pool,
            tc.tile_pool(name="g4_pool", bufs=3) as g4_pool,
            tc.tile_pool(name="tmp_pool", bufs=2) as tmp_pool,
            tc.tile_pool(name="sm_pool", bufs=4) as sm,
            tc.tile_pool(name="ps_mt", bufs=2, space="PSUM") as ps_mt,
            tc.tile_pool(name="ps_sm", bufs=3, space="PSUM") as ps_sm,
            tc.tile_pool(name="ps_ua", bufs=1, space="PSUM") as ps_ua,
        ):
            # ---- per-core statics ----
            ones_f = singles.tile([1, 256], f32)
            nc.vector.memset(ones_f, 1.0)
            ones_bf = singles.tile([128, 1], bf16)
            nc.vector.memset(ones_bf, 1.0)
            ident_f = singles.tile([128, 128], f32)
            nc.sync.dma_start(out=ident_f, in_=idn[:, :])
            ident_bf = singles.tile([128, 128], bf16)
            nc.vector.tensor_copy(ident_bf, ident_f)
            uwt_sb = singles.tile([128, C, Q + 1], f32)
            nc.sync.dma_start(
                out=uwt_sb, in_=uwt.rearrange("(c p) q -> p c q", p=128)
            )
            usm_sb = singles.tile([1, Q + 1], f32)
            nc.sync.dma_start(out=usm_sb, in_=usm[:, :])
            u_f = singles.tile([Q, D], f32)
            nc.sync.dma_start(out=u_f, in_=uu[:, :])
            u_bf = singles.tile([Q, D], bf16)
            nc.vector.tensor_copy(u_bf, u_f)
            hm_sb = singles.tile([1, S * P], f32)
            nc.sync.dma_start(out=hm_sb, in_=hmf[:, :])

            for s in range(S):
                # ---- load hT[s]: [768,256] -> [128, 6, 256] (d on partitions)
                ht_sb = ht_pool.tile([128, C, 256], f32)
                nc.sync.dma_start(
                    out=ht_sb, in_=ht[s].rearrange("(c p) q -> p c q", p=128)
                )

                # ---- logits MT_ext [97, 256]: rows 0:96 = logits+usm, row 96 = hs
                mt = ps_mt.tile([Q + 1, 256], f32, tag="psmt")
                for c in range(C):
                    nc.tensor.matmul(
                        mt,
                        lhsT=uwt_sb[:, c, :],
                        rhs=ht_sb[:, c, :],
                        start=(c == 0),
                        stop=False,
                    )
                nc.tensor.matmul(
                    mt, lhsT=usm_sb, rhs=ones_f[:, 0:256], start=False, stop=True
                )

                # ---- E = exp(logits) [96,256] bf16; hs row -> sbuf
                e_sb = e_pool.tile([Q, 256], bf16)
                nc.scalar.activation(e_sb, mt[0:Q, :], AF.Exp)
                hs_row = sm.tile([1, 256], f32)
                nc.scalar.copy(hs_row, mt[Q : Q + 1, :])

                # ---- Zq[p] = sum_q E   (ones matmul), then 1/Zq via exp(-ln)
                zq = ps_sm.tile([1, 256], f32, tag="pssm")
                nc.tensor.matmul(zq, lhsT=ones_bf[0:Q, :], rhs=e_sb)
                lnzq = sm.tile([1, 256], f32)
                nc.scalar.activation(lnzq, zq, AF.Ln)
                zqr = sm.tile([1, 256], f32)
                nc.scalar.activation(zqr, lnzq, AF.Exp, scale=-1.0)

                # ---- broadcast 1/Zq over q partitions; c2q = E * (1/Zq)
                zb = ps_sm.tile([Q, 256], f32, tag="pssm")
                nc.tensor.matmul(zb, lhsT=ones_f[0:1, 0:Q], rhs=zqr)
                c2q = c2q_pool.tile([Q, 256], bf16)
                nc.vector.tensor_mul(c2q, e_sb, zb)

                # ---- u_aT[d,p] = sum_q u[q,d] c2q[q,p]  -> psum [128, 6, 256]
                ua = ps_ua.tile([128, C, 256], f32)
                for c in range(C):
                    nc.tensor.matmul(
                        ua[:, c, :],
                        lhsT=u_bf[:, c * 128 : (c + 1) * 128],
                        rhs=c2q,
                    )
                g2 = g2_pool.tile([128, C, 256], f32)
                nc.scalar.copy(g2, ua)
                g3 = g3_pool.tile([128, C, 256], f32)
                nc.vector.tensor_mul(g3, ht_sb, g2)

                # ---- rmax path: transpose E halves -> [128, 2, 96], max over q
                te = ps_mt.tile([128, 2, Q], bf16, tag="psmt")
                nc.tensor.transpose(
                    te[:, 0, :], e_sb[:, 0:128], ident_bf[0:Q, 0:Q]
                )
                nc.tensor.transpose(
                    te[:, 1, :], e_sb[:, 128:256], ident_bf[0:Q, 0:Q]
                )
                m_col2 = sm.tile([128, 2], f32)
                nc.vector.tensor_reduce(m_col2, te, axis=AX.X, op=ALU.max)
                mrow = ps_sm.tile([1, 256], f32, tag="pssm")
                nc.tensor.transpose(
                    mrow[0:1, 0:128], m_col2[:, 0:1], ident_f
                )
                nc.tensor.transpose(
                    mrow[0:1, 128:256], m_col2[:, 1:2], ident_f
                )

                # ---- q2c over p (single-partition row ops)
                t0 = sm.tile([1, 256], f32)
                nc.vector.tensor_add(
                    t0, hs_row, hm_sb[:, s * 256 : (s + 1) * 256]
                )
                xrow = sm.tile([1, 256], f32)
                nc.scalar.activation(xrow, t0, AF.Exp)
                erow = sm.tile([1, 256], f32)
                nc.vector.tensor_mul(erow, xrow, mrow)
                zp = sm.tile([1, 1], f32)
                nc.vector.tensor_reduce(zp, erow, axis=AX.X, op=ALU.add)
                rzp = sm.tile([1, 1], f32)
                nc.vector.reciprocal(rzp, zp)
                q2c = sm.tile([1, 256], f32)
                nc.vector.tensor_scalar_mul(q2c, in0=erow, scalar1=rzp)

                # ---- h_aT[d] = sum_p hT[d,p] q2c[p]
                qb = ps_sm.tile([128, 256], f32, tag="pssm")
                nc.tensor.matmul(qb, lhsT=ones_f[0:1, 0:128], rhs=q2c)
                qb_sb = sm.tile([128, 256], f32)
                nc.scalar.copy(qb_sb, qb)
                tmp = tmp_pool.tile([128, C, 256], f32)
                for c in range(C):
                    nc.gpsimd.tensor_mul(tmp[:, c, :], ht_sb[:, c, :], qb_sb)
                ha_col = sm.tile([128, C], f32)
                nc.vector.tensor_reduce(ha_col, tmp, axis=AX.X, op=ALU.add)

                # ---- g4 = hT * h_a (per-partition scalar per chunk)
                g4 = g4_pool.tile([128, C, 256], f32)
                for c in range(C):
                    nc.vector.tensor_scalar_mul(
                        g4[:, c, :],
                        in0=ht_sb[:, c, :],
                        scalar1=ha_col[:, c : c + 1],
                    )

                # ---- outputs
                def oview(k):
                    return out[s, k * D : (k + 1) * D, :].rearrange(
                        "(c p) q -> p c q", p=128
                    )

                nc.sync.dma_start(out=oview(0), in_=ht_sb)
                nc.sync.dma_start(out=oview(1), in_=g2)
                nc.sync.dma_start(out=oview(2), in_=g3)
                nc.sync.dma_start(out=oview(3), in_=g4)

    nc.compile()
    return nc


def _get_nc():
    global _NC
    if _NC is None:
        _NC = _build_nc()
    return _NC


def kernel(h, u, h_mask, u_mask, is_train=0, w=None, b=None):
    global LAST_EXEC_NS
    h = np.asarray(h, dtype=np.float32)
    u = np.asarray(u, dtype=np.float32)
    h_mask = np.asarray(h_mask, dtype=np.float32)
    u_mask = np.asarray(u_mask, dtype=np.float32)
    w = np.asarray(w, dtype=np.float32)

    w_h, w_u, w_hu = w[:D], w[D : 2 * D], w[2 * D :]

    # host-side prep (tiny)
    hT = np.ascontiguousarray(h.transpose(0, 1, 3, 2))  # [B,S,D,P]
    uw = u * w_hu[None, None, :]  # [B,Q,D]
    uwt = np.empty((B, D, Q + 1), dtype=np.float32)
    uwt[:, :, :Q] = uw.transpose(0, 2, 1)
    uwt[:, :, Q] = w_h[None, :]
    usm = np.zeros((B, 1, Q + 1), dtype=np.float32)
    usm[:, 0, :Q] = u @ w_u + (u_mask - 1.0) * NEG
    hmneg = ((h_mask - 1.0) * NEG).reshape(B, 1, S * P).astype(np.float32)
    ident = np.eye(128, dtype=np.float32)

    in_maps = [
        {
            "ht": hT[i],
            "uwt": uwt[i],
            "usm": usm[i],
            "u": u[i],
            "hmneg": hmneg[i],
            "ident": ident,
        }
        for i in range(NCORES)
    ]

    from concourse.bass_utils import run_bass_kernel_spmd

    nc = _get_nc()
    res = run_bass_kernel_spmd(
        nc, in_maps, core_ids=list(range(NCORES)), trace=_TRACE
    )
    LAST_EXEC_NS = res.exec_time_ns
    globals()["LAST_RESULT"] = res

    gT = np.stack([res.results[i]["out"] for i in range(NCORES)])  # [B,S,4D,P]
    g = np.ascontiguousarray(gT.transpose(0, 1, 3, 2))  # [B,S,P,4D]
    return g
